# revision 1
# baseline (speedup 1.0000x reference)
"""Trainium2 Bass kernel for nn_MultiHeadAttention_18700287607660.

Math (B=128, L=500, D=512, NWAY=5, n_head=1):
  qp = q@Wq.T ; kp = k@Wk.T ; vp = v@Wv.T
  attn_avg = softmax(mean_over_groups(qp @ kp.T / temp))     # [B, 5, L]
  proto = attn_avg @ vp                                      # [B, 5, D]
  out1 = LN1(broadcast(proto) + kp)
  out  = LN2(leaky_relu(out1@Wfc.T, 0.1) + out1)

Key restructurings (exact up to fp reassociation):
  * mean happens BEFORE softmax, so the [500,500] attention matrix is never
    formed:  S = (Sel @ q) @ (Wq.T @ Wk / temp) @ k.T   with Sel the [5,500]
    group-mean selector. Wqk = Wq.T@Wk/temp folded on the host.
  * proto = (A @ v) @ Wv.T  — the V projection is never materialized.
  * broadcast(proto) is a K=5 matmul accumulated straight into kp's PSUM.

Implementation choices:
  * matmul operands in fp16 — fp32 moving operands stream at half rate on the
    PE; fp16 keeps ~1e-3 accuracy with fp32 PSUM accumulation.
  * seq dim host-padded 500->512 (zero rows) so the DMA xbar transpose engine
    (2-byte dtypes, rows%16==0) produces k^T and x^T — no PE transposes or
    PSUM->SBUF copybacks for the two big transposed tensors.
  * LayerNorm rstd via exp(-0.5*ln(var+eps)): keeps every ACT function in the
    one "natural_log_exp_and_others" table set (one table load total).

Sharding: pure data parallel, 16 batches per core across 8 cores.
"""
import os
import sys

for _p in ("/opt/trn_rl_repo", "/root/.axon_site/_ro/trn_rl_repo"):
    if os.path.isdir(_p) and _p not in sys.path:
        sys.path.insert(0, _p)

import numpy as np

import concourse.bacc as bacc
import concourse.bass as bass
import concourse.tile as tile
from concourse import mybir
from concourse.bass_utils import run_bass_kernel_spmd

F16 = mybir.dt.float16
F32 = mybir.dt.float32
N_CORES = 8
B = 128
BPC = B // N_CORES   # 16 batches per core
L = 500              # true seq len
LP = 512             # padded seq len (DMA xbar transpose needs rows%16==0)
LT = 128             # l-tile
NLT = LP // LT       # 4
LTAIL = L - 3 * LT   # 116 valid rows in the last l-tile
D = 512
DT = 128
NDT = D // DT        # 4
W = 5                # NWAY shot groups
TEMP = float(np.sqrt(float(D)))
EPS = 1e-6
LEAK = 0.1

# All ACT functions used here (Exp, Ln, Relu, Copy, Identity) live in the
# "natural_log_exp_and_others" table set, but bacc's per-activation greedy
# set chooser still flips between sets (hundreds of ~2.7us ACT_TABLE_LOADs).
# Empty out every other set (keeping positions, since act_func_set_id is the
# positional index into act_info.json) so exactly one set is ever loaded.
_orig_get_activation_tables = bacc.get_activation_tables


def _pinned_activation_tables(module_arch):
    tables = _orig_get_activation_tables(module_arch)
    if "natural_log_exp_and_others" in tables:
        return {
            name: (fns if name == "natural_log_exp_and_others" else set())
            for name, fns in tables.items()
        }
    return tables


bacc.get_activation_tables = _pinned_activation_tables


def _emit(nc, tc, ext, apply_gb):
    """Software-pipelined emission: per iteration s we emit
    load(s+2) | attn(s+1) | kp+LN1+xT(s) | fc+LN2+store(s-1)
    so every stage's inputs were produced in a previous iteration and the
    per-engine streams always have ready work from an adjacent batch.
    """
    import contextlib
    ctx = contextlib.ExitStack()
    with ctx:
        const = ctx.enter_context(tc.tile_pool(name="const", bufs=1))
        pin = ctx.enter_context(tc.tile_pool(name="pin", bufs=3))
        pkt = ctx.enter_context(tc.tile_pool(name="pkt", bufs=4))
        px = ctx.enter_context(tc.tile_pool(name="px", bufs=3))
        pxt = ctx.enter_context(tc.tile_pool(name="pxt", bufs=3))
        pt = ctx.enter_context(tc.tile_pool(name="pt", bufs=3))
        pr = ctx.enter_context(tc.tile_pool(name="pr", bufs=3))
        po = ctx.enter_context(tc.tile_pool(name="po", bufs=3))
        tiny = ctx.enter_context(tc.tile_pool(name="tiny", bufs=3))
        ptiny = ctx.enter_context(tc.tile_pool(name="ptiny", bufs=3))
        ps_small = ctx.enter_context(tc.tile_pool(name="ps_small", bufs=2, space="PSUM"))
        ps_kp = ctx.enter_context(tc.tile_pool(name="ps_kp", bufs=4, space="PSUM"))
        ps_fc = ctx.enter_context(tc.tile_pool(name="ps_fc", bufs=2, space="PSUM"))

        # ---- constants ----
        wkT_sb = const.tile([DT, NDT, D], F16)
        wvT_sb = const.tile([DT, NDT, D], F16)
        wfcT_sb = const.tile([DT, NDT, D], F16)
        wqk_sb = const.tile([DT, NDT, D], F16)
        for w_sb, name in ((wkT_sb, "wkT"), (wvT_sb, "wvT"),
                           (wfcT_sb, "wfcT"), (wqk_sb, "wqk")):
            nc.sync.dma_start(out=w_sb, in_=ext[name].rearrange("(i p) e -> p i e", p=DT))
        selT_sb = const.tile([LT, NLT, W], F16)
        nc.sync.dma_start(out=selT_sb, in_=ext["selT"].rearrange("(i p) w -> p i w", p=LT))
        bc5_sb = const.tile([DT, NLT, LT], F16)
        nc.sync.dma_start(out=bc5_sb, in_=ext["bc5"].rearrange("w (i p) -> w i p", p=LT))
        id_sb = const.tile([W, W], F16)
        nc.sync.dma_start(out=id_sb, in_=ext["ident"][:])
        eps_sb = const.tile([DT, 1], F32)
        nc.vector.memset(eps_sb, EPS)
        gb_sb = {}
        if apply_gb:
            for name in ("g1", "b1", "g2", "b2"):
                t = const.tile([LT, D], F32)
                src = ext[name]
                bcast = bass.AP(tensor=src.tensor, offset=src.offset,
                                ap=[[0, LT]] + list(src.ap))
                nc.sync.dma_start(out=t, in_=bcast)
                gb_sb[name] = t

        state = {}

        def stage_load(b):
            st = state.setdefault(b, {})
            qv = pin.tile([LT, 2, NLT, D], F16, tag="qv", name=f"qv{b}")
            st["q"] = qv[:, 0, :, :]
            st["v"] = qv[:, 1, :, :]
            st["kT"] = pkt.tile([DT, NDT, LP], F16, tag="kT", name=f"kT{b}")
            nc.sync.dma_start(out=qv, in_=ext["qv"][b].rearrange("t (i p) d -> p t i d", p=LT))
            nc.sync.dma_start_transpose(out=st["kT"], in_=ext["k"][b])

        def stage_attn(b):
            st = state[b]
            q_sb, v_sb, kT_sb = st["q"], st["v"], st["kT"]
            psq = ps_small.tile([W, D], F32, tag="small")
            for i in range(NLT):
                nc.tensor.matmul(psq, lhsT=selT_sb[:, i, :], rhs=q_sb[:, i, :],
                                 start=(i == 0), stop=(i == NLT - 1))
            qb_sb = tiny.tile([W, D], F16, tag="qb")
            nc.scalar.copy(out=qb_sb, in_=psq)
            ptr5 = ps_small.tile([DT, NDT, 8], F16, tag="small")
            for i in range(NDT):
                nc.tensor.transpose(ptr5[:, i, :W], qb_sb[:, i * DT:(i + 1) * DT], id_sb)
            qbT_sb = tiny.tile([DT, NDT, W], F16, tag="qbT")
            nc.vector.tensor_copy(out=qbT_sb, in_=ptr5[:, :, :W])

            pqk = ps_small.tile([W, D], F32, tag="small")
            for i in range(NDT):
                nc.tensor.matmul(pqk, lhsT=qbT_sb[:, i, :], rhs=wqk_sb[:, i, :],
                                 start=(i == 0), stop=(i == NDT - 1))
            qk_sb = tiny.tile([W, D], F16, tag="qk")
            nc.scalar.copy(out=qk_sb, in_=pqk)
            ptrq = ps_small.tile([DT, NDT, 8], F16, tag="small")
            for i in range(NDT):
                nc.tensor.transpose(ptrq[:, i, :W], qk_sb[:, i * DT:(i + 1) * DT], id_sb)
            qkT_sb = tiny.tile([DT, NDT, W], F16, tag="qkT")
            nc.vector.tensor_copy(out=qkT_sb, in_=ptrq[:, :, :W])

            pS = ps_small.tile([W, L], F32, tag="small")
            for i in range(NDT):
                nc.tensor.matmul(pS, lhsT=qkT_sb[:, i, :], rhs=kT_sb[:, i, :L],
                                 start=(i == 0), stop=(i == NDT - 1))

            negmax = tiny.tile([W, 1], F32, tag="negmax")
            nc.vector.tensor_reduce(out=negmax, in_=pS, axis=mybir.AxisListType.X,
                                    op=mybir.AluOpType.max, negate=True)
            E_sb = tiny.tile([W, LP], F16, tag="E")
            sume = tiny.tile([W, 1], F32, tag="sume")
            nc.scalar.activation(out=E_sb[:, :L], in_=pS,
                                 func=mybir.ActivationFunctionType.Exp,
                                 bias=negmax, scale=1.0, accum_out=sume)
            nc.vector.memset(E_sb[:, L:], 0.0)
            rcp = tiny.tile([W, 1], F32, tag="rcp")
            nc.vector.reciprocal(out=rcp, in_=sume)
            A_sb = tiny.tile([W, LP], F16, tag="A")
            nc.vector.tensor_scalar_mul(out=A_sb, in0=E_sb, scalar1=rcp)

            ptrA = ps_small.tile([LT, NLT, 8], F16, tag="small")
            for i in range(NLT):
                nc.tensor.transpose(ptrA[:, i, :W], A_sb[:, i * LT:(i + 1) * LT], id_sb)
            AT_sb = tiny.tile([LT, NLT, W], F16, tag="AT")
            nc.vector.tensor_copy(out=AT_sb, in_=ptrA[:, :, :W])

            pt1 = ps_small.tile([W, D], F32, tag="small")
            for i in range(NLT):
                nc.tensor.matmul(pt1, lhsT=AT_sb[:, i, :], rhs=v_sb[:, i, :],
                                 start=(i == 0), stop=(i == NLT - 1))
            t1_sb = tiny.tile([W, D], F16, tag="t1")
            nc.scalar.copy(out=t1_sb, in_=pt1)
            ptrt = ps_small.tile([DT, NDT, 8], F16, tag="small")
            for i in range(NDT):
                nc.tensor.transpose(ptrt[:, i, :W], t1_sb[:, i * DT:(i + 1) * DT], id_sb)
            t1T_sb = tiny.tile([DT, NDT, DT], F16, tag="t1T")
            nc.vector.memset(t1T_sb, 0.0)
            # replicate t1T's 5 columns into 4x 32-col groups (write AP [32x4, 1x5])
            rd = ptrt[:, :, :W]
            rep_in = bass.AP(tensor=rd.tensor, offset=rd.offset,
                             ap=[list(rd.ap[0]), list(rd.ap[1]), [0, 4], list(rd.ap[2])])
            wr = t1T_sb
            rep_out = bass.AP(tensor=wr.tensor, offset=wr.offset,
                              ap=[list(wr.ap[0]), list(wr.ap[1]), [32, 4], [1, W]])
            nc.vector.tensor_copy(out=rep_out, in_=rep_in)

            ppr = ps_small.tile([DT, D], F32, tag="small")
            for i in range(NDT):
                nc.tensor.matmul(ppr, lhsT=t1T_sb[:, i, :], rhs=wvT_sb[:, i, :],
                                 start=(i == 0), stop=(i == NDT - 1))
            proto_sb = ptiny.tile([DT, D], F16, tag="proto")
            nc.scalar.copy(out=proto_sb, in_=ppr)
            st["proto"] = proto_sb

        def stage_kp(b):
            st = state[b]
            kT_sb, proto_sb = st["kT"], st["proto"]
            x_sb = px.tile([LT, NLT, D], F16, tag="x")
            xT_sb = pxt.tile([DT, NDT, LP], F16, tag="xT")
            st1 = tiny.tile([LT, NLT, 6], F32, tag="st1")
            mv1 = tiny.tile([LT, NLT, 2], F32, tag="mv1")
            u1 = tiny.tile([LT, NLT], F32, tag="u1")
            rstd1 = tiny.tile([LT, NLT], F32, tag="rstd1")
            nb1 = tiny.tile([LT, NLT], F32, tag="nb1")
            pkps = {}
            for lt in range(NLT):
                pkp = ps_kp.tile([LT, D], F32, tag="kp", name=f"kp{b}_{lt}")
                pkps[lt] = pkp
                for dt in range(NDT):
                    nc.tensor.matmul(pkp, lhsT=kT_sb[:, dt, lt * LT:(lt + 1) * LT],
                                     rhs=wkT_sb[:, dt, :], start=(dt == 0), stop=False)
            for lt in range(NLT):
                nc.tensor.matmul(pkps[lt], lhsT=bc5_sb[32 * lt:32 * lt + W, lt, :],
                                 rhs=proto_sb[32 * lt:32 * lt + W, :],
                                 start=False, stop=True, tile_position=(32 * lt, 0))
            for lt in range(NLT):
                pkp = pkps[lt]
                nc.vector.bn_stats(out=st1[:, lt, :], in_=pkp)
                nc.vector.bn_aggr(out=mv1[:, lt, :], in_=st1[:, lt, :])
                nc.scalar.activation(out=u1[:, lt:lt + 1], in_=mv1[:, lt, 1:2],
                                     func=mybir.ActivationFunctionType.Ln,
                                     bias=eps_sb, scale=1.0)
                nc.scalar.activation(out=rstd1[:, lt:lt + 1], in_=u1[:, lt:lt + 1],
                                     func=mybir.ActivationFunctionType.Exp,
                                     bias=0.0, scale=-0.5)
                nc.vector.scalar_tensor_tensor(out=nb1[:, lt:lt + 1],
                                               in0=mv1[:, lt, 0:1], scalar=-1.0,
                                               in1=rstd1[:, lt:lt + 1],
                                               op0=mybir.AluOpType.mult,
                                               op1=mybir.AluOpType.mult)
                nc.scalar.activation(out=x_sb[:, lt, :], in_=pkps[lt],
                                     func=mybir.ActivationFunctionType.Identity,
                                     bias=nb1[:, lt:lt + 1],
                                     scale=rstd1[:, lt:lt + 1])
                if apply_gb:
                    nc.vector.tensor_mul(out=x_sb[:, lt, :], in0=x_sb[:, lt, :],
                                         in1=gb_sb["g1"])
                    nc.vector.tensor_add(out=x_sb[:, lt, :], in0=x_sb[:, lt, :],
                                         in1=gb_sb["b1"])
                nc.sync.dma_start_transpose(out=xT_sb[:, :, lt * LT:(lt + 1) * LT],
                                            in_=x_sb[:, lt, :])
            st["x"] = x_sb
            st["xT"] = xT_sb

        def stage_fc(b):
            st = state[b]
            x_sb, xT_sb = st["x"], st["xT"]
            t_sb = pt.tile([LT, NLT, D], F32, tag="t")
            r_sb = pr.tile([LT, NLT, D], F32, tag="r")
            o_sb = po.tile([LT, NLT, D], F16, tag="o")
            st2 = tiny.tile([LT, NLT, 6], F32, tag="st2")
            mv2 = tiny.tile([LT, NLT, 2], F32, tag="mv2")
            u2 = tiny.tile([LT, NLT], F32, tag="u2")
            rstd2 = tiny.tile([LT, NLT], F32, tag="rstd2")
            for lt in range(NLT):
                py = ps_fc.tile([LT, D], F32, tag="fc")
                for et in range(NDT):
                    nc.tensor.matmul(py, lhsT=xT_sb[:, et, lt * LT:(lt + 1) * LT],
                                     rhs=wfcT_sb[:, et, :],
                                     start=(et == 0), stop=(et == NDT - 1))
                a_lt = t_sb[:, lt, :]
                # leaky(z) = z + (1-LEAK)*relu(-z)
                nc.scalar.activation(out=a_lt, in_=py,
                                     func=mybir.ActivationFunctionType.Relu,
                                     bias=0.0, scale=-1.0)
                nc.vector.scalar_tensor_tensor(out=a_lt, in0=a_lt, scalar=(1.0 - LEAK),
                                               in1=py, op0=mybir.AluOpType.mult,
                                               op1=mybir.AluOpType.add)
                nc.gpsimd.tensor_add(out=r_sb[:, lt, :], in0=a_lt, in1=x_sb[:, lt, :])
                nc.vector.bn_stats(out=st2[:, lt, :], in_=r_sb[:, lt, :])
                nc.vector.bn_aggr(out=mv2[:, lt, :], in_=st2[:, lt, :])
            nc.scalar.activation(out=u2, in_=mv2[:, :, 1],
                                 func=mybir.ActivationFunctionType.Ln,
                                 bias=eps_sb, scale=1.0)
            nc.scalar.activation(out=rstd2, in_=u2,
                                 func=mybir.ActivationFunctionType.Exp,
                                 bias=0.0, scale=-0.5)
            for lt in range(NLT):
                nc.vector.tensor_scalar(out=o_sb[:, lt, :], in0=r_sb[:, lt, :],
                                        scalar1=mv2[:, lt, 0:1],
                                        scalar2=rstd2[:, lt:lt + 1],
                                        op0=mybir.AluOpType.subtract,
                                        op1=mybir.AluOpType.mult)
                if apply_gb:
                    nc.vector.tensor_mul(out=o_sb[:, lt, :], in0=o_sb[:, lt, :],
                                         in1=gb_sb["g2"])
                    nc.vector.tensor_add(out=o_sb[:, lt, :], in0=o_sb[:, lt, :],
                                         in1=gb_sb["b2"])
            nc.sync.dma_start(out=ext["out"][b].rearrange("(i p) d -> p i d", p=LT),
                              in_=o_sb)
            del state[b]


        def stage_kp_fc_interleaved(bk, bf):
            stage_kp(bk)
            stage_fc(bf)
        # pipelined emission
        stage_load(0)
        stage_load(1)
        stage_attn(0)
        for s in range(BPC):
            if s + 2 < BPC:
                stage_load(s + 2)
            if s + 1 < BPC:
                stage_attn(s + 1)
            if s >= 1:
                stage_kp_fc_interleaved(s, s - 1)
            else:
                stage_kp(s)
        stage_fc(BPC - 1)


_PROGRAM_CACHE = {}


def _build(apply_gb):
    key = bool(apply_gb)
    if key in _PROGRAM_CACHE:
        return _PROGRAM_CACHE[key]
    nc = bacc.Bacc("TRN2", target_bir_lowering=False, debug=False,
                   num_devices=N_CORES)
    ext = {}
    ext["qv"] = nc.declare_dram_parameter("qv", [BPC, 2, LP, D], F16, isOutput=False)
    ext["k"] = nc.declare_dram_parameter("k", [BPC, LP, D], F16, isOutput=False)
    for name in ("wkT", "wvT", "wfcT", "wqk"):
        ext[name] = nc.declare_dram_parameter(name, [D, D], F16, isOutput=False)
    ext["ident"] = nc.declare_dram_parameter("ident", [W, W], F16, isOutput=False)
    ext["selT"] = nc.declare_dram_parameter("selT", [LP, W], F16, isOutput=False)
    ext["bc5"] = nc.declare_dram_parameter("bc5", [DT, LP], F16, isOutput=False)
    if apply_gb:
        for name in ("g1", "b1", "g2", "b2"):
            ext[name] = nc.declare_dram_parameter(name, [D], F32, isOutput=False)
    ext["out"] = nc.declare_dram_parameter("out", [BPC, LP, D], F16, isOutput=True)

    with tile.TileContext(nc) as tc:
        _emit(nc, tc, ext, apply_gb)
    nc.compile()
    _PROGRAM_CACHE[key] = (nc, apply_gb)
    return _PROGRAM_CACHE[key]


def kernel(q, k, v, Wq, Wk, Wv, Wfc, g1, b1, g2, b2, _trace=False):
    q = np.asarray(q, dtype=np.float32)
    k = np.asarray(k, dtype=np.float32)
    v = np.asarray(v, dtype=np.float32)
    Wq = np.asarray(Wq, dtype=np.float32)
    Wk = np.asarray(Wk, dtype=np.float32)
    Wv = np.asarray(Wv, dtype=np.float32)
    Wfc = np.asarray(Wfc, dtype=np.float32)
    g1 = np.asarray(g1, dtype=np.float32)
    b1 = np.asarray(b1, dtype=np.float32)
    g2 = np.asarray(g2, dtype=np.float32)
    b2 = np.asarray(b2, dtype=np.float32)

    apply_gb = not (np.all(g1 == 1) and np.all(b1 == 0)
                    and np.all(g2 == 1) and np.all(b2 == 0))

    def pad16(x):
        out = np.zeros((BPC * N_CORES, LP, D), dtype=np.float16)
        out[:, :L, :] = x.astype(np.float16)
        return out

    q16, k16, v16 = pad16(q), pad16(k), pad16(v)
    qv16 = np.ascontiguousarray(np.stack([q16, v16], axis=1))
    wkT = np.ascontiguousarray(Wk.T).astype(np.float16)
    wvT = np.ascontiguousarray(Wv.T).astype(np.float16)
    wfcT = np.ascontiguousarray(Wfc.T).astype(np.float16)
    wqk = ((Wq.T.astype(np.float64) @ Wk.astype(np.float64)) / TEMP).astype(np.float16)
    ident = np.eye(W, dtype=np.float16)
    sel = np.zeros((LP, W), dtype=np.float16)
    sel[np.arange(L), np.arange(L) % W] = np.float16(W / L)
    bc5 = np.zeros((DT, LP), dtype=np.float16)
    for _l in range(L):
        bc5[32 * (_l // LT) + _l % W, _l] = 1.0

    nc, _ = _build(apply_gb)

    in_maps = []
    for c in range(N_CORES):
        m = {
            "qv": qv16[c * BPC:(c + 1) * BPC],
            "k": k16[c * BPC:(c + 1) * BPC],
            "wkT": wkT, "wvT": wvT, "wfcT": wfcT, "wqk": wqk,
            "ident": ident, "selT": sel, "bc5": bc5,
        }
        if apply_gb:
            m.update({"g1": g1, "b1": b1, "g2": g2, "b2": b2})
        in_maps.append(m)

    res = run_bass_kernel_spmd(nc, in_maps, core_ids=list(range(N_CORES)),
                               trace=_trace)
    out = np.concatenate([res.results[c]["out"] for c in range(N_CORES)], axis=0)[:, :L, :].astype(np.float32)
    if _trace:
        kernel._last_results = res
    return out



# revision 30
# speedup vs baseline: 1.6655x; 1.6655x over previous
"""Trainium2 Bass kernel for nn_MultiHeadAttention_18700287607660.

Math (B=128, L=500, D=512, NWAY=5, n_head=1):
  qp = q@Wq.T ; kp = k@Wk.T ; vp = v@Wv.T
  attn_avg = softmax(mean_over_groups(qp @ kp.T / temp))     # [B, 5, L]
  proto = attn_avg @ vp                                      # [B, 5, D]
  out1 = LN1(broadcast(proto) + kp)
  out  = LN2(leaky_relu(out1@Wfc.T, 0.1) + out1)

Restructurings (exact up to fp reassociation):
  * mean before softmax: S = (Sel@q) @ (Wq.T@Wk/temp) @ k.T, Wqk host-folded.
  * the whole attention chain is computed TRANSPOSED ([d, 5] layouts) so every
    matmul has a 5-wide moving dim: the PE cost model charges out-free-size
    cycles, so these are ~4ns each instead of ~213ns.  The softmax norm is
    deferred: sums come from a [5,1] ones-matmul, 1/sum is applied as a
    per-partition Act scale on the final [5,512] proto copy.
  * softmax max-subtraction dropped (logits ~N(0, 0.1^2) — exp is safe); the
    1/100 group-mean scale rides the exp's input scale.
  * proto'v broadcast over shots is a K=5 matmul accumulated into kp's PSUM.
  * leaky_relu is a single fused Prelu activation (alpha=0.1).
  * q, v stream in as fp8e4m3 (attention-side only — error lands on proto,
    which is ~5% the magnitude of kp); k stays fp16 and is transposed on the
    host so the kernel does a plain load instead of a DMA-xbar transpose.

Sharding: pure data parallel, 16 batches per core across 8 cores.
"""
import os
import sys

for _p in ("/opt/trn_rl_repo", "/root/.axon_site/_ro/trn_rl_repo"):
    if os.path.isdir(_p) and _p not in sys.path:
        sys.path.insert(0, _p)

import numpy as np
import ml_dtypes

import concourse.bacc as bacc
import concourse.bass as bass
import concourse.tile as tile
from concourse import mybir
from concourse.bass_utils import run_bass_kernel_spmd

F8 = mybir.dt.float8e4
F16 = mybir.dt.float16
F32 = mybir.dt.float32
ACT = mybir.ActivationFunctionType
N_CORES = 8
B = 128
BPC = B // N_CORES   # 16 batches per core
L = 500              # true seq len
LP = 512             # padded seq len
LT = 128             # l-tile
NLT = LP // LT       # 4
D = 512
DT = 128
NDT = D // DT        # 4
W = 5                # NWAY shot groups
TEMP = float(np.sqrt(float(D)))
NG = 100.0           # number of shot groups averaged (Lq // NWAY)
EPS = 1e-6
LEAK = 0.1

# All ACT functions used here (Exp, Ln, Prelu, Copy, Identity) live in the
# "natural_log_exp_and_others" table set, but bacc's per-activation greedy
# set chooser still flips between sets.  Empty out every other set (keeping
# positions, since act_func_set_id is the positional index) so exactly one
# set is ever loaded.
_orig_get_activation_tables = bacc.get_activation_tables


def _pinned_activation_tables(module_arch):
    tables = _orig_get_activation_tables(module_arch)
    if "natural_log_exp_and_others" in tables:
        return {
            name: (fns if name == "natural_log_exp_and_others" else set())
            for name, fns in tables.items()
        }
    return tables


bacc.get_activation_tables = _pinned_activation_tables


def _emit(nc, tc, ext, apply_gb):
    """Software-pipelined emission.  Iteration s emits
        store(s-2) | load(s+2) | attn(s+1) x y(s) x fc(s-1)
    with the attention chain's tiny matmuls interleaved between the big GEMM
    l-tiles so the in-order PE stream never waits on a cross-engine hop.
    """
    import contextlib
    ctx = contextlib.ExitStack()
    with ctx:
        const = ctx.enter_context(tc.tile_pool(name="const", bufs=1))
        pqv = ctx.enter_context(tc.tile_pool(name="pqv", bufs=4))
        pkt = ctx.enter_context(tc.tile_pool(name="pkt", bufs=4))
        px = ctx.enter_context(tc.tile_pool(name="px", bufs=5))
        pxt = ctx.enter_context(tc.tile_pool(name="pxt", bufs=4))
        pt = ctx.enter_context(tc.tile_pool(name="pt", bufs=3))
        pr = ctx.enter_context(tc.tile_pool(name="pr", bufs=3))
        po = ctx.enter_context(tc.tile_pool(name="po", bufs=2))
        tiny = ctx.enter_context(tc.tile_pool(name="tiny", bufs=3))
        ps_y = ctx.enter_context(tc.tile_pool(name="ps_y", bufs=4, space="PSUM"))
        ps_z = ctx.enter_context(tc.tile_pool(name="ps_z", bufs=3, space="PSUM"))
        ps_att = ctx.enter_context(tc.tile_pool(name="ps_att", bufs=1, space="PSUM"))

        # ---- constants ----
        wkT_sb = const.tile([DT, NDT, D], F16)
        wfcT_sb = const.tile([DT, NDT, D], F16)
        wvT_sb = const.tile([DT, NDT, D], F16)
        wqk_sb = const.tile([DT, NDT, D], F16)
        sel_sb = const.tile([LT, NLT, W], F8)
        nc.sync.dma_start(out=sel_sb, in_=ext["sel"].rearrange("(i p) w -> p i w", p=LT))
        ones_sb = const.tile([LT, NLT, 1], F8)
        nc.sync.dma_start(out=ones_sb, in_=ext["ones"].rearrange("(i p) o -> p i o", p=LT))
        bc5_sb = const.tile([W, LP], F16)
        nc.sync.dma_start(out=bc5_sb, in_=ext["bc5"][:])
        id_sb = const.tile([DT, DT], F16)
        nc.sync.dma_start(out=id_sb, in_=ext["ident"][:])
        # big weights loaded in first-use order, AFTER the first batch's q/v/k
        # loads are queued (see the prologue below) so attn(0) starts early.
        eps_sb = const.tile([DT, 1], F32)
        nc.vector.memset(eps_sb, EPS)
        gb_sb = {}
        if apply_gb:
            for name in ("g1", "b1", "g2", "b2"):
                t = const.tile([LT, D], F32)
                src = ext[name]
                bcast = bass.AP(tensor=src.tensor, offset=src.offset,
                                ap=[[0, LT]] + list(src.ap))
                nc.sync.dma_start(out=t, in_=bcast)
                gb_sb[name] = t

        state = {}

        def load(b):
            st = state.setdefault(b, {})
            st["qv"] = pqv.tile([LT, 2, NLT, D], F8, tag="qv", name=f"qv{b}")
            st["kT"] = pkt.tile([DT, NDT, LP], F16, tag="kT", name=f"kT{b}")
            nc.gpsimd.dma_start(out=st["qv"],
                                in_=ext["qv"][b].rearrange("t (i p) d -> p t i d", p=LT))
            nc.sync.dma_start(out=st["kT"],
                              in_=ext["kT"][b].rearrange("(i p) l -> p i l", p=DT))

        # ---- attention chain, transposed; split into interleavable steps ----
        def attn_qbar(b):
            st = state[b]
            psA = ps_att.tile([DT, NDT, W], F32, tag="att")
            q = st["qv"][:, 0, :, :]
            for j in range(NDT):
                for lt in range(NLT):
                    nc.tensor.matmul(psA[:, j, :], lhsT=q[:, lt, j * DT:(j + 1) * DT],
                                     rhs=sel_sb[:, lt, :],
                                     start=(lt == 0), stop=(lt == NLT - 1))
            st["qbT"] = tiny.tile([DT, NDT, W], F16, tag="qbT", name=f"qbT{b}")
            nc.vector.tensor_copy(out=st["qbT"], in_=psA)

        def attn_qk(b):
            st = state[b]
            psB = ps_att.tile([DT, NDT, W], F32, tag="att")
            for j in range(NDT):
                for i in range(NDT):
                    nc.tensor.matmul(psB[:, j, :], lhsT=wqk_sb[:, i, j * DT:(j + 1) * DT],
                                     rhs=st["qbT"][:, i, :],
                                     start=(i == 0), stop=(i == NDT - 1))
            st["qkT"] = tiny.tile([DT, NDT, W], F16, tag="qkT", name=f"qkT{b}")
            nc.vector.tensor_copy(out=st["qkT"], in_=psB)

        def attn_S(b):
            st = state[b]
            psS = ps_att.tile([LT, NLT, W], F32, tag="att")
            for lt in range(NLT):
                for i in range(NDT):
                    nc.tensor.matmul(psS[:, lt, :],
                                     lhsT=st["kT"][:, i, lt * LT:(lt + 1) * LT],
                                     rhs=st["qkT"][:, i, :],
                                     start=(i == 0), stop=(i == NDT - 1))
            # E = exp(S_raw / NG); the 1/NG group-mean scale rides the act.
            st["E"] = tiny.tile([LT, NLT, W], F8, tag="E", name=f"E{b}")
            nc.scalar.activation(out=st["E"], in_=psS, func=ACT.Exp,
                                 bias=0.0, scale=1.0 / NG)

        def attn_sums(b):
            st = state[b]
            psSum = ps_att.tile([W, 1], F32, tag="att")
            for lt in range(NLT):
                nc.tensor.matmul(psSum, lhsT=st["E"][:, lt, :],
                                 rhs=ones_sb[:, lt, :],
                                 start=(lt == 0), stop=(lt == NLT - 1))
            st["rcp"] = tiny.tile([W, 1], F32, tag="rcp", name=f"rcp{b}")
            nc.vector.reciprocal(out=st["rcp"], in_=psSum)

        def attn_t1(b):
            st = state[b]
            psC = ps_att.tile([DT, NDT, W], F32, tag="att")
            v = st["qv"][:, 1, :, :]
            for j in range(NDT):
                for lt in range(NLT):
                    nc.tensor.matmul(psC[:, j, :], lhsT=v[:, lt, j * DT:(j + 1) * DT],
                                     rhs=st["E"][:, lt, :],
                                     start=(lt == 0), stop=(lt == NLT - 1))
            st["t1T"] = tiny.tile([DT, NDT, W], F16, tag="t1T", name=f"t1T{b}")
            nc.vector.tensor_copy(out=st["t1T"], in_=psC)

        def attn_pv(b):
            st = state[b]
            psD = ps_att.tile([DT, NDT, W], F32, tag="att")
            for j in range(NDT):
                for i in range(NDT):
                    nc.tensor.matmul(psD[:, j, :], lhsT=wvT_sb[:, i, j * DT:(j + 1) * DT],
                                     rhs=st["t1T"][:, i, :],
                                     start=(i == 0), stop=(i == NDT - 1))
            st["pvT"] = tiny.tile([DT, NDT, W], F16, tag="pvT", name=f"pvT{b}")
            nc.vector.tensor_copy(out=st["pvT"], in_=psD)

        def attn_proto(b):
            st = state[b]
            psP = ps_att.tile([W, D], F16, tag="att")
            for j in range(NDT):
                nc.tensor.transpose(psP[:, j * DT:(j + 1) * DT], st["pvT"][:, j, :], id_sb)
            st["protoV"] = tiny.tile([W, D], F16, tag="protoV", name=f"protoV{b}")
            nc.scalar.activation(out=st["protoV"], in_=psP, func=ACT.Copy,
                                 scale=st["rcp"])

        # ---- y = kp + bcast(proto); LN1 ----
        def y_gemm(b, lt):
            st = state[b]
            if lt == 0:
                st["psY"] = {}
                st["st1"] = tiny.tile([LT, NLT, 6], F32, tag="st1", name=f"st1_{b}")
                st["mv1"] = tiny.tile([LT, NLT, 2], F32, tag="mv1", name=f"mv1_{b}")
            psY = ps_y.tile([LT, D], F32, tag="y", name=f"y{b}_{lt}")
            st["psY"][lt] = psY
            for i in range(NDT):
                nc.tensor.matmul(psY, lhsT=st["kT"][:, i, lt * LT:(lt + 1) * LT],
                                 rhs=wkT_sb[:, i, :], start=(i == 0), stop=False)
            nc.tensor.matmul(psY, lhsT=bc5_sb[:, lt * LT:(lt + 1) * LT],
                             rhs=st["protoV"], start=False, stop=True)
            nc.vector.bn_stats(out=st["st1"][:, lt, :], in_=psY)
            nc.vector.bn_aggr(out=st["mv1"][:, lt, :], in_=st["st1"][:, lt, :])

        def y_finish(b, lt):
            # Per-l-tile LN1 tail: frees the y PSUM bank early — the cycle
            # "y(b,lt) gemm -> stats -> rstd -> apply1 -> free bank for
            # y(b+1,lt)" is the loop-carried constraint that paces the whole
            # pipeline, so it must not wait for the other l-tiles' stats.
            st = state[b]
            mv1 = st["mv1"]
            if lt == 0:
                st["u1"] = tiny.tile([LT, NLT], F32, tag="u1", name=f"u1_{b}")
                st["rstd1"] = tiny.tile([LT, NLT], F32, tag="rstd1", name=f"rstd1_{b}")
                st["nb1"] = tiny.tile([LT, NLT], F32, tag="nb1", name=f"nb1_{b}")
                st["x"] = px.tile([LT, NLT, D], F16, tag="x", name=f"x{b}")
                st["xT"] = pxt.tile([DT, NLT, NDT, LT], F16, tag="xT", name=f"xT{b}")
            u1, rstd1, nb1 = st["u1"], st["rstd1"], st["nb1"]
            nc.scalar.activation(out=u1[:, lt:lt + 1], in_=mv1[:, lt, 1:2],
                                 func=ACT.Ln, bias=eps_sb, scale=1.0)
            nc.scalar.activation(out=rstd1[:, lt:lt + 1], in_=u1[:, lt:lt + 1],
                                 func=ACT.Exp, bias=0.0, scale=-0.5)
            nc.vector.scalar_tensor_tensor(out=nb1[:, lt:lt + 1],
                                           in0=mv1[:, lt, 0:1], scalar=-1.0,
                                           in1=rstd1[:, lt:lt + 1],
                                           op0=mybir.AluOpType.mult,
                                           op1=mybir.AluOpType.mult)
            nc.scalar.activation(out=st["x"][:, lt, :], in_=st["psY"][lt],
                                 func=ACT.Identity,
                                 bias=nb1[:, lt:lt + 1], scale=rstd1[:, lt:lt + 1])
            if apply_gb:
                nc.vector.tensor_mul(out=st["x"][:, lt, :], in0=st["x"][:, lt, :],
                                     in1=gb_sb["g1"])
                nc.vector.tensor_add(out=st["x"][:, lt, :], in0=st["x"][:, lt, :],
                                     in1=gb_sb["b1"])
            del st["psY"][lt]
            if lt == NLT - 1:
                # one xbar transpose for the whole batch: [128, 2048] in and
                # out are fully contiguous with this xT layout.
                nc.sync.dma_start_transpose(out=st["xT"], in_=st["x"])

        # ---- fc + leaky + residual; LN2 ----
        def fc_gemm(b, lt):
            st = state[b]
            if lt == 0:
                st["t"] = pt.tile([LT, NLT, D], F16, tag="t", name=f"t{b}")
                st["r"] = pr.tile([LT, NLT, D], F16, tag="r", name=f"r{b}")
                st["st2"] = tiny.tile([LT, NLT, 6], F32, tag="st2", name=f"st2_{b}")
                st["mv2"] = tiny.tile([LT, NLT, 2], F32, tag="mv2", name=f"mv2_{b}")
            psZ = ps_z.tile([LT, D], F32, tag="z")
            for i in range(NDT):
                nc.tensor.matmul(psZ, lhsT=st["xT"][:, lt, i, :],
                                 rhs=wfcT_sb[:, i, :],
                                 start=(i == 0), stop=(i == NDT - 1))
            nc.scalar.activation(out=st["t"][:, lt, :], in_=psZ, func=ACT.Prelu,
                                 alpha=LEAK)
            eng = nc.vector if lt % 2 == 0 else nc.gpsimd
            eng.tensor_add(out=st["r"][:, lt, :], in0=st["t"][:, lt, :],
                           in1=st["x"][:, lt, :])
            nc.vector.bn_stats(out=st["st2"][:, lt, :], in_=st["r"][:, lt, :])
            nc.vector.bn_aggr(out=st["mv2"][:, lt, :], in_=st["st2"][:, lt, :])

        def fc_finish(b):
            st = state[b]
            mv2 = st["mv2"]
            u2 = tiny.tile([LT, NLT], F32, tag="u2")
            rstd2 = tiny.tile([LT, NLT], F32, tag="rstd2")
            nc.scalar.activation(out=u2, in_=mv2[:, :, 1], func=ACT.Ln,
                                 bias=eps_sb, scale=1.0)
            nc.scalar.activation(out=rstd2, in_=u2, func=ACT.Exp,
                                 bias=0.0, scale=-0.5)
            o = po.tile([LT, NLT, D], F16, tag="o", name=f"o{b}")
            for lt in range(NLT):
                nc.vector.tensor_scalar(out=o[:, lt, :], in0=st["r"][:, lt, :],
                                        scalar1=mv2[:, lt, 0:1],
                                        scalar2=rstd2[:, lt:lt + 1],
                                        op0=mybir.AluOpType.subtract,
                                        op1=mybir.AluOpType.mult)
                if apply_gb:
                    nc.vector.tensor_mul(out=o[:, lt, :], in0=o[:, lt, :],
                                         in1=gb_sb["g2"])
                    nc.vector.tensor_add(out=o[:, lt, :], in0=o[:, lt, :],
                                         in1=gb_sb["b2"])
            st["o"] = o

        def store(b):
            nc.gpsimd.dma_start(out=ext["out"][b].rearrange("(i p) d -> p i d", p=LT),
                                in_=state[b]["o"])
            del state[b]

        # ---- pipelined emission ----
        st0 = state.setdefault(0, {})
        st0["qv"] = pqv.tile([LT, 2, NLT, D], F8, tag="qv", name="qv0")
        st0["kT"] = pkt.tile([DT, NDT, LP], F16, tag="kT", name="kT0")
        nc.gpsimd.dma_start(out=st0["qv"],
                            in_=ext["qv"][0].rearrange("t (i p) d -> p t i d", p=LT))
        nc.sync.dma_start(out=wqk_sb, in_=ext["wqk"].rearrange("(i p) e -> p i e", p=DT))
        nc.sync.dma_start(out=st0["kT"],
                          in_=ext["kT"][0].rearrange("(i p) l -> p i l", p=DT))
        nc.sync.dma_start(out=wvT_sb, in_=ext["wvT"].rearrange("(i p) e -> p i e", p=DT))
        nc.sync.dma_start(out=wkT_sb, in_=ext["wkT"].rearrange("(i p) e -> p i e", p=DT))
        nc.sync.dma_start(out=wfcT_sb, in_=ext["wfcT"].rearrange("(i p) e -> p i e", p=DT))
        load(1)
        attn_qbar(0)
        attn_qk(0)
        attn_S(0)
        attn_sums(0)
        attn_t1(0)
        attn_pv(0)
        attn_proto(0)
        for s in range(BPC + 3):
            a = s + 1 if s + 1 < BPC else None
            b = s if s < BPC else None
            f = s - 2 if 2 <= s <= BPC + 1 else None
            g = s - 3 if s >= 3 else None
            if a is not None:
                attn_qbar(a)
            if g is not None:
                fc_finish(g)
            if s + 2 < BPC:
                load(s + 2)
            if b is not None:
                y_gemm(b, 0)
            if a is not None:
                attn_qk(a)
            if b is not None:
                y_gemm(b, 1)
                y_finish(b, 0)
            if a is not None:
                attn_S(a)
            if b is not None:
                y_gemm(b, 2)
                y_finish(b, 1)
            if a is not None:
                attn_sums(a)
            if b is not None:
                y_gemm(b, 3)
                y_finish(b, 2)
            if a is not None:
                attn_t1(a)
            if b is not None:
                y_finish(b, 3)
            if f is not None:
                fc_gemm(f, 0)
            if a is not None:
                attn_pv(a)
            if f is not None:
                fc_gemm(f, 1)
            if g is not None:
                store(g)
            if a is not None:
                attn_proto(a)
            if f is not None:
                fc_gemm(f, 2)
                fc_gemm(f, 3)


_PROGRAM_CACHE = {}


def _build(apply_gb):
    key = bool(apply_gb)
    if key in _PROGRAM_CACHE:
        return _PROGRAM_CACHE[key]
    nc = bacc.Bacc("TRN2", target_bir_lowering=False, debug=False,
                   num_devices=N_CORES)
    ext = {}
    ext["qv"] = nc.declare_dram_parameter("qv", [BPC, 2, LP, D], F8, isOutput=False)
    ext["kT"] = nc.declare_dram_parameter("kT", [BPC, D, LP], F16, isOutput=False)
    for name in ("wkT", "wfcT", "wvT", "wqk"):
        ext[name] = nc.declare_dram_parameter(name, [D, D], F16, isOutput=False)
    ext["sel"] = nc.declare_dram_parameter("sel", [LP, W], F8, isOutput=False)
    ext["ones"] = nc.declare_dram_parameter("ones", [LP, 1], F8, isOutput=False)
    ext["bc5"] = nc.declare_dram_parameter("bc5", [W, LP], F16, isOutput=False)
    ext["ident"] = nc.declare_dram_parameter("ident", [DT, DT], F16, isOutput=False)
    if apply_gb:
        for name in ("g1", "b1", "g2", "b2"):
            ext[name] = nc.declare_dram_parameter(name, [D], F32, isOutput=False)
    ext["out"] = nc.declare_dram_parameter("out", [BPC, LP, D], F16, isOutput=True)

    with tile.TileContext(nc) as tc:
        _emit(nc, tc, ext, apply_gb)
    nc.compile()
    _PROGRAM_CACHE[key] = (nc, apply_gb)
    return _PROGRAM_CACHE[key]


def kernel(q, k, v, Wq, Wk, Wv, Wfc, g1, b1, g2, b2, _trace=False):
    q = np.asarray(q, dtype=np.float32)
    k = np.asarray(k, dtype=np.float32)
    v = np.asarray(v, dtype=np.float32)
    Wq = np.asarray(Wq, dtype=np.float32)
    Wk = np.asarray(Wk, dtype=np.float32)
    Wv = np.asarray(Wv, dtype=np.float32)
    Wfc = np.asarray(Wfc, dtype=np.float32)
    g1 = np.asarray(g1, dtype=np.float32)
    b1 = np.asarray(b1, dtype=np.float32)
    g2 = np.asarray(g2, dtype=np.float32)
    b2 = np.asarray(b2, dtype=np.float32)

    apply_gb = not (np.all(g1 == 1) and np.all(b1 == 0)
                    and np.all(g2 == 1) and np.all(b2 == 0))

    f8 = ml_dtypes.float8_e4m3

    def pad8(x):
        out = np.zeros((B, LP, D), dtype=f8)
        out[:, :L, :] = x.astype(f8)
        return out

    qv8 = np.ascontiguousarray(np.stack([pad8(q), pad8(v)], axis=1))
    k16 = np.zeros((B, LP, D), dtype=np.float16)
    k16[:, :L, :] = k.astype(np.float16)
    kT16 = np.ascontiguousarray(k16.transpose(0, 2, 1))
    wkT = np.ascontiguousarray(Wk.T).astype(np.float16)
    wfcT = np.ascontiguousarray(Wfc.T).astype(np.float16)
    wvT = np.ascontiguousarray(Wv.T).astype(np.float16)
    wqk = ((Wq.T.astype(np.float64) @ Wk.astype(np.float64)) / TEMP).astype(np.float16)
    sel = np.zeros((LP, W), dtype=f8)
    sel[np.arange(L), np.arange(L) % W] = 1.0
    ones = np.zeros((LP, 1), dtype=f8)
    ones[:L] = 1.0
    bc5 = np.zeros((W, LP), dtype=np.float16)
    bc5[np.arange(L) % W, np.arange(L)] = 1.0
    ident = np.eye(DT, dtype=np.float16)

    nc, _ = _build(apply_gb)

    in_maps = []
    for c in range(N_CORES):
        m = {
            "qv": qv8[c * BPC:(c + 1) * BPC],
            "kT": kT16[c * BPC:(c + 1) * BPC],
            "wkT": wkT, "wfcT": wfcT, "wvT": wvT, "wqk": wqk,
            "sel": sel, "ones": ones, "bc5": bc5, "ident": ident,
        }
        if apply_gb:
            m.update({"g1": g1, "b1": b1, "g2": g2, "b2": b2})
        in_maps.append(m)

    res = run_bass_kernel_spmd(nc, in_maps, core_ids=list(range(N_CORES)),
                               trace=_trace)
    out = np.concatenate([res.results[c]["out"] for c in range(N_CORES)],
                         axis=0)[:, :L, :].astype(np.float32)
    if _trace:
        kernel._last_results = res
    return out


# revision 40
# speedup vs baseline: 1.6891x; 1.0142x over previous
"""Trainium2 Bass kernel for nn_MultiHeadAttention_18700287607660.

Math (B=128, L=500, D=512, NWAY=5, n_head=1):
  qp = q@Wq.T ; kp = k@Wk.T ; vp = v@Wv.T
  attn_avg = softmax(mean_over_groups(qp @ kp.T / temp))     # [B, 5, L]
  proto = attn_avg @ vp                                      # [B, 5, D]
  out1 = LN1(broadcast(proto) + kp)
  out  = LN2(leaky_relu(out1@Wfc.T, 0.1) + out1)

Restructurings (exact up to fp reassociation):
  * mean before softmax: S = (Sel@q) @ (Wq.T@Wk/temp) @ k.T, Wqk host-folded.
  * the whole attention chain is computed TRANSPOSED ([d, 5] layouts) so every
    matmul has a 5-wide moving dim: the PE cost model charges out-free-size
    cycles, so these are ~4ns each instead of ~213ns.  The softmax norm is
    deferred: sums come from a [5,1] ones-matmul, 1/sum is applied as a
    per-partition Act scale on the final [5,512] proto copy.
  * softmax max-subtraction dropped (logits ~N(0, 0.1^2) — exp is safe); the
    1/100 group-mean scale rides the exp's input scale.
  * proto'v broadcast over shots is a K=5 matmul accumulated into kp's PSUM.
  * leaky_relu is a single fused Prelu activation (alpha=0.1).
  * q, v stream in as fp8e4m3 (attention-side only — error lands on proto,
    which is ~5% the magnitude of kp); k stays fp16 and is transposed on the
    host so the kernel does a plain load instead of a DMA-xbar transpose.

Sharding: pure data parallel, 16 batches per core across 8 cores.
"""
import os
import sys

for _p in ("/opt/trn_rl_repo", "/root/.axon_site/_ro/trn_rl_repo"):
    if os.path.isdir(_p) and _p not in sys.path:
        sys.path.insert(0, _p)

import numpy as np
import ml_dtypes

import concourse.bacc as bacc
import concourse.bass as bass
import concourse.tile as tile
from concourse import mybir
from concourse.bass_utils import run_bass_kernel_spmd

F8 = mybir.dt.float8e4
F16 = mybir.dt.float16
F32 = mybir.dt.float32
ACT = mybir.ActivationFunctionType
N_CORES = 8
B = 128
BPC = B // N_CORES   # 16 batches per core
L = 500              # true seq len
LP = 512             # padded seq len
LT = 128             # l-tile
NLT = LP // LT       # 4
D = 512
DT = 128
NDT = D // DT        # 4
W = 5                # NWAY shot groups
TEMP = float(np.sqrt(float(D)))
NG = 100.0           # number of shot groups averaged (Lq // NWAY)
EPS = 1e-6
LEAK = 0.1

# All ACT functions used here (Exp, Ln, Prelu, Copy, Identity) live in the
# "natural_log_exp_and_others" table set, but bacc's per-activation greedy
# set chooser still flips between sets.  Empty out every other set (keeping
# positions, since act_func_set_id is the positional index) so exactly one
# set is ever loaded.
_orig_get_activation_tables = bacc.get_activation_tables


def _pinned_activation_tables(module_arch):
    tables = _orig_get_activation_tables(module_arch)
    if "natural_log_exp_and_others" in tables:
        return {
            name: (fns if name == "natural_log_exp_and_others" else set())
            for name, fns in tables.items()
        }
    return tables


bacc.get_activation_tables = _pinned_activation_tables


def _emit(nc, tc, ext, apply_gb):
    """Software-pipelined emission.  Iteration s emits
        store(s-2) | load(s+2) | attn(s+1) x y(s) x fc(s-1)
    with the attention chain's tiny matmuls interleaved between the big GEMM
    l-tiles so the in-order PE stream never waits on a cross-engine hop.
    """
    import contextlib
    ctx = contextlib.ExitStack()
    with ctx:
        const = ctx.enter_context(tc.tile_pool(name="const", bufs=1))
        pqv = ctx.enter_context(tc.tile_pool(name="pqv", bufs=4))
        pkt = ctx.enter_context(tc.tile_pool(name="pkt", bufs=4))
        px = ctx.enter_context(tc.tile_pool(name="px", bufs=5))
        pxt = ctx.enter_context(tc.tile_pool(name="pxt", bufs=4))
        pt = ctx.enter_context(tc.tile_pool(name="pt", bufs=3))
        pr = ctx.enter_context(tc.tile_pool(name="pr", bufs=3))
        po = ctx.enter_context(tc.tile_pool(name="po", bufs=2))
        tiny = ctx.enter_context(tc.tile_pool(name="tiny", bufs=3))
        ps_y = ctx.enter_context(tc.tile_pool(name="ps_y", bufs=4, space="PSUM"))
        ps_z = ctx.enter_context(tc.tile_pool(name="ps_z", bufs=3, space="PSUM"))
        ps_att = ctx.enter_context(tc.tile_pool(name="ps_att", bufs=1, space="PSUM"))

        # ---- constants ----
        wkT_sb = const.tile([DT, NDT, D], F16)
        wfcT_sb = const.tile([DT, NDT, D], F16)
        wvT_sb = const.tile([DT, NDT, D], F16)
        wqk_sb = const.tile([DT, NDT, D], F16)
        sel_sb = const.tile([LT, NLT, W], F8)
        nc.sync.dma_start(out=sel_sb, in_=ext["sel"].rearrange("(i p) w -> p i w", p=LT))
        ones_sb = const.tile([LT, NLT, 1], F8)
        nc.sync.dma_start(out=ones_sb, in_=ext["ones"].rearrange("(i p) o -> p i o", p=LT))
        bc5_sb = const.tile([W, LP], F16)
        nc.sync.dma_start(out=bc5_sb, in_=ext["bc5"][:])
        id_sb = const.tile([DT, DT], F16)
        nc.sync.dma_start(out=id_sb, in_=ext["ident"][:])
        # big weights loaded in first-use order, AFTER the first batch's q/v/k
        # loads are queued (see the prologue below) so attn(0) starts early.
        eps_sb = const.tile([DT, 1], F32)
        nc.vector.memset(eps_sb, EPS)
        gb_sb = {}
        if apply_gb:
            for name in ("g1", "b1", "g2", "b2"):
                t = const.tile([LT, D], F32)
                src = ext[name]
                bcast = bass.AP(tensor=src.tensor, offset=src.offset,
                                ap=[[0, LT]] + list(src.ap))
                nc.sync.dma_start(out=t, in_=bcast)
                gb_sb[name] = t

        state = {}

        def load(b):
            st = state.setdefault(b, {})
            st["qv"] = pqv.tile([LT, 2, NLT, D], F8, tag="qv", name=f"qv{b}")
            st["kT"] = pkt.tile([DT, NDT, LP], F16, tag="kT", name=f"kT{b}")
            nc.gpsimd.dma_start(out=st["qv"],
                                in_=ext["qv"][b].rearrange("t (i p) d -> p t i d", p=LT))
            nc.sync.dma_start(out=st["kT"],
                              in_=ext["kT"][b].rearrange("(i p) l -> p i l", p=DT))

        # ---- attention chain, transposed; split into interleavable steps ----
        def attn_qbar(b):
            st = state[b]
            psA = ps_att.tile([DT, NDT, W], F32, tag="att")
            q = st["qv"][:, 0, :, :]
            for j in range(NDT):
                for lt in range(NLT):
                    nc.tensor.matmul(psA[:, j, :], lhsT=q[:, lt, j * DT:(j + 1) * DT],
                                     rhs=sel_sb[:, lt, :],
                                     start=(lt == 0), stop=(lt == NLT - 1))
            st["qbT"] = tiny.tile([DT, NDT, W], F16, tag="qbT", name=f"qbT{b}")
            nc.vector.tensor_copy(out=st["qbT"], in_=psA)

        def attn_qk(b):
            st = state[b]
            psB = ps_att.tile([DT, NDT, W], F32, tag="att")
            for j in range(NDT):
                for i in range(NDT):
                    nc.tensor.matmul(psB[:, j, :], lhsT=wqk_sb[:, i, j * DT:(j + 1) * DT],
                                     rhs=st["qbT"][:, i, :],
                                     start=(i == 0), stop=(i == NDT - 1))
            st["qkT"] = tiny.tile([DT, NDT, W], F16, tag="qkT", name=f"qkT{b}")
            nc.vector.tensor_copy(out=st["qkT"], in_=psB)

        def attn_S(b):
            st = state[b]
            psS = ps_att.tile([LT, NLT, W], F32, tag="att")
            for lt in range(NLT):
                for i in range(NDT):
                    nc.tensor.matmul(psS[:, lt, :],
                                     lhsT=st["kT"][:, i, lt * LT:(lt + 1) * LT],
                                     rhs=st["qkT"][:, i, :],
                                     start=(i == 0), stop=(i == NDT - 1))
            # E = exp(S_raw / NG); the 1/NG group-mean scale rides the act.
            st["E"] = tiny.tile([LT, NLT, W], F8, tag="E", name=f"E{b}")
            nc.scalar.activation(out=st["E"], in_=psS, func=ACT.Exp,
                                 bias=0.0, scale=1.0 / NG)

        def attn_sums(b):
            st = state[b]
            psSum = ps_att.tile([W, 1], F32, tag="att")
            for lt in range(NLT):
                nc.tensor.matmul(psSum, lhsT=st["E"][:, lt, :],
                                 rhs=ones_sb[:, lt, :],
                                 start=(lt == 0), stop=(lt == NLT - 1))
            st["rcp"] = tiny.tile([W, 1], F32, tag="rcp", name=f"rcp{b}")
            nc.vector.reciprocal(out=st["rcp"], in_=psSum)

        def attn_t1(b):
            st = state[b]
            psC = ps_att.tile([DT, NDT, W], F32, tag="att")
            v = st["qv"][:, 1, :, :]
            for j in range(NDT):
                for lt in range(NLT):
                    nc.tensor.matmul(psC[:, j, :], lhsT=v[:, lt, j * DT:(j + 1) * DT],
                                     rhs=st["E"][:, lt, :],
                                     start=(lt == 0), stop=(lt == NLT - 1))
            st["t1T"] = tiny.tile([DT, NDT, W], F16, tag="t1T", name=f"t1T{b}")
            nc.vector.tensor_copy(out=st["t1T"], in_=psC)

        def attn_pv(b):
            st = state[b]
            psD = ps_att.tile([DT, NDT, W], F32, tag="att")
            for j in range(NDT):
                for i in range(NDT):
                    nc.tensor.matmul(psD[:, j, :], lhsT=wvT_sb[:, i, j * DT:(j + 1) * DT],
                                     rhs=st["t1T"][:, i, :],
                                     start=(i == 0), stop=(i == NDT - 1))
            st["pvT"] = tiny.tile([DT, NDT, W], F16, tag="pvT", name=f"pvT{b}")
            nc.vector.tensor_copy(out=st["pvT"], in_=psD)

        def attn_proto(b):
            st = state[b]
            psP = ps_att.tile([W, D], F16, tag="att")
            for j in range(NDT):
                nc.tensor.transpose(psP[:, j * DT:(j + 1) * DT], st["pvT"][:, j, :], id_sb)
            st["protoV"] = tiny.tile([W, D], F16, tag="protoV", name=f"protoV{b}")
            nc.scalar.activation(out=st["protoV"], in_=psP, func=ACT.Copy,
                                 scale=st["rcp"])

        # ---- y = kp + bcast(proto); LN1 ----
        def y_gemm(b, lt):
            st = state[b]
            if lt == 0:
                st["psY"] = {}
                st["st1"] = tiny.tile([LT, NLT, 6], F32, tag="st1", name=f"st1_{b}")
                st["mv1"] = tiny.tile([LT, NLT, 2], F32, tag="mv1", name=f"mv1_{b}")
            psY = ps_y.tile([LT, D], F32, tag="y", name=f"y{b}_{lt}")
            st["psY"][lt] = psY
            for i in range(NDT):
                nc.tensor.matmul(psY, lhsT=st["kT"][:, i, lt * LT:(lt + 1) * LT],
                                 rhs=wkT_sb[:, i, :], start=(i == 0), stop=False)
            nc.tensor.matmul(psY, lhsT=bc5_sb[:, lt * LT:(lt + 1) * LT],
                             rhs=st["protoV"], start=False, stop=True)
            nc.vector.bn_stats(out=st["st1"][:, lt, :], in_=psY)
            nc.vector.bn_aggr(out=st["mv1"][:, lt, :], in_=st["st1"][:, lt, :])

        def y_finish(b, lt):
            # Per-l-tile LN1 tail: frees the y PSUM bank early — the cycle
            # "y(b,lt) gemm -> stats -> rstd -> apply1 -> free bank for
            # y(b+1,lt)" is the loop-carried constraint that paces the whole
            # pipeline, so it must not wait for the other l-tiles' stats.
            st = state[b]
            mv1 = st["mv1"]
            if lt == 0:
                st["u1"] = tiny.tile([LT, NLT], F32, tag="u1", name=f"u1_{b}")
                st["rstd1"] = tiny.tile([LT, NLT], F32, tag="rstd1", name=f"rstd1_{b}")
                st["nb1"] = tiny.tile([LT, NLT], F32, tag="nb1", name=f"nb1_{b}")
                st["x"] = px.tile([LT, NLT, D], F16, tag="x", name=f"x{b}")
                st["xT"] = pxt.tile([DT, NLT, NDT, LT], F16, tag="xT", name=f"xT{b}")
            u1, rstd1, nb1 = st["u1"], st["rstd1"], st["nb1"]
            nc.scalar.activation(out=u1[:, lt:lt + 1], in_=mv1[:, lt, 1:2],
                                 func=ACT.Ln, bias=eps_sb, scale=1.0)
            nc.scalar.activation(out=rstd1[:, lt:lt + 1], in_=u1[:, lt:lt + 1],
                                 func=ACT.Exp, bias=0.0, scale=-0.5)
            nc.vector.scalar_tensor_tensor(out=nb1[:, lt:lt + 1],
                                           in0=mv1[:, lt, 0:1], scalar=-1.0,
                                           in1=rstd1[:, lt:lt + 1],
                                           op0=mybir.AluOpType.mult,
                                           op1=mybir.AluOpType.mult)
            nc.scalar.activation(out=st["x"][:, lt, :], in_=st["psY"][lt],
                                 func=ACT.Identity,
                                 bias=nb1[:, lt:lt + 1], scale=rstd1[:, lt:lt + 1])
            if apply_gb:
                nc.vector.tensor_mul(out=st["x"][:, lt, :], in0=st["x"][:, lt, :],
                                     in1=gb_sb["g1"])
                nc.vector.tensor_add(out=st["x"][:, lt, :], in0=st["x"][:, lt, :],
                                     in1=gb_sb["b1"])
            del st["psY"][lt]
            if b == BPC - 1 and lt in (1, NLT - 1):
                # tail: nothing left to overlap, so ship xT in halves to let
                # the last fc start sooner.
                nc.sync.dma_start_transpose(out=st["xT"][:, lt - 1:lt + 1, :, :],
                                            in_=st["x"][:, lt - 1:lt + 1, :])
            elif b != BPC - 1 and lt == NLT - 1:
                # one xbar transpose for the whole batch: [128, 2048] in and
                # out are fully contiguous with this xT layout.
                nc.sync.dma_start_transpose(out=st["xT"], in_=st["x"])

        # ---- fc + leaky + residual; LN2 ----
        def fc_gemm(b, lt):
            st = state[b]
            if lt == 0:
                st["t"] = pt.tile([LT, NLT, D], F16, tag="t", name=f"t{b}")
                st["r"] = pr.tile([LT, NLT, D], F16, tag="r", name=f"r{b}")
                st["st2"] = tiny.tile([LT, NLT, 6], F32, tag="st2", name=f"st2_{b}")
                st["mv2"] = tiny.tile([LT, NLT, 2], F32, tag="mv2", name=f"mv2_{b}")
            psZ = ps_z.tile([LT, D], F32, tag="z")
            for i in range(NDT):
                nc.tensor.matmul(psZ, lhsT=st["xT"][:, lt, i, :],
                                 rhs=wfcT_sb[:, i, :],
                                 start=(i == 0), stop=(i == NDT - 1))
            nc.scalar.activation(out=st["t"][:, lt, :], in_=psZ, func=ACT.Prelu,
                                 alpha=LEAK)
            eng = nc.vector if (lt % 2 == 0 or b == BPC - 1) else nc.gpsimd
            eng.tensor_add(out=st["r"][:, lt, :], in0=st["t"][:, lt, :],
                           in1=st["x"][:, lt, :])
            nc.vector.bn_stats(out=st["st2"][:, lt, :], in_=st["r"][:, lt, :])
            nc.vector.bn_aggr(out=st["mv2"][:, lt, :], in_=st["st2"][:, lt, :])
            if b == BPC - 1:
                # tail: telescoped per-l-tile LN2 finish + store — nothing
                # overlaps the last batch, so latency is all that matters.
                if lt == 0:
                    st["u2"] = tiny.tile([LT, NLT], F32, tag="u2t", name="u2_tail")
                    st["rstd2"] = tiny.tile([LT, NLT], F32, tag="rstd2t",
                                            name="rstd2_tail")
                    st["o"] = po.tile([LT, NLT, D], F16, tag="o", name=f"o{b}")
                nc.scalar.activation(out=st["u2"][:, lt:lt + 1],
                                     in_=st["mv2"][:, lt, 1:2],
                                     func=ACT.Ln, bias=eps_sb, scale=1.0)
                nc.scalar.activation(out=st["rstd2"][:, lt:lt + 1],
                                     in_=st["u2"][:, lt:lt + 1],
                                     func=ACT.Exp, bias=0.0, scale=-0.5)
                nc.vector.tensor_scalar(out=st["o"][:, lt, :],
                                        in0=st["r"][:, lt, :],
                                        scalar1=st["mv2"][:, lt, 0:1],
                                        scalar2=st["rstd2"][:, lt:lt + 1],
                                        op0=mybir.AluOpType.subtract,
                                        op1=mybir.AluOpType.mult)
                if apply_gb:
                    nc.vector.tensor_mul(out=st["o"][:, lt, :],
                                         in0=st["o"][:, lt, :], in1=gb_sb["g2"])
                    nc.vector.tensor_add(out=st["o"][:, lt, :],
                                         in0=st["o"][:, lt, :], in1=gb_sb["b2"])
                nc.sync.dma_start(
                    out=ext["out"][b].rearrange("(i p) d -> p i d", p=LT)[:, lt, :],
                    in_=st["o"][:, lt, :])

        def fc_finish(b):
            st = state[b]
            mv2 = st["mv2"]
            u2 = tiny.tile([LT, NLT], F32, tag="u2")
            rstd2 = tiny.tile([LT, NLT], F32, tag="rstd2")
            nc.scalar.activation(out=u2, in_=mv2[:, :, 1], func=ACT.Ln,
                                 bias=eps_sb, scale=1.0)
            nc.scalar.activation(out=rstd2, in_=u2, func=ACT.Exp,
                                 bias=0.0, scale=-0.5)
            o = po.tile([LT, NLT, D], F16, tag="o", name=f"o{b}")
            for lt in range(NLT):
                nc.vector.tensor_scalar(out=o[:, lt, :], in0=st["r"][:, lt, :],
                                        scalar1=mv2[:, lt, 0:1],
                                        scalar2=rstd2[:, lt:lt + 1],
                                        op0=mybir.AluOpType.subtract,
                                        op1=mybir.AluOpType.mult)
                if apply_gb:
                    nc.vector.tensor_mul(out=o[:, lt, :], in0=o[:, lt, :],
                                         in1=gb_sb["g2"])
                    nc.vector.tensor_add(out=o[:, lt, :], in0=o[:, lt, :],
                                         in1=gb_sb["b2"])
            st["o"] = o

        def store(b):
            eng = nc.sync if b == BPC - 1 else nc.gpsimd
            eng.dma_start(out=ext["out"][b].rearrange("(i p) d -> p i d", p=LT),
                          in_=state[b]["o"])
            del state[b]

        # ---- pipelined emission ----
        load(0)
        for w_sb, name in ((wqk_sb, "wqk"), (wkT_sb, "wkT"),
                           (wvT_sb, "wvT"), (wfcT_sb, "wfcT")):
            nc.sync.dma_start(out=w_sb, in_=ext[name].rearrange("(i p) e -> p i e", p=DT))
        load(1)
        attn_qbar(0)
        attn_qk(0)
        attn_S(0)
        attn_sums(0)
        attn_t1(0)
        attn_pv(0)
        attn_proto(0)
        for s in range(BPC + 3):
            a = s + 1 if s + 1 < BPC else None
            b = s if s < BPC else None
            f = s - 2 if 2 <= s <= BPC + 1 else None
            g = s - 3 if s >= 3 else None
            if a is not None:
                attn_qbar(a)
            if g is not None and g < BPC - 1:
                fc_finish(g)
            if s + 2 < BPC:
                load(s + 2)
            if b is not None:
                y_gemm(b, 0)
            if a is not None:
                attn_qk(a)
            if b is not None:
                y_gemm(b, 1)
                y_finish(b, 0)
            if a is not None:
                attn_S(a)
            if b is not None:
                y_gemm(b, 2)
                y_finish(b, 1)
            if a is not None:
                attn_sums(a)
            if b is not None:
                y_gemm(b, 3)
                y_finish(b, 2)
            if a is not None:
                attn_t1(a)
            if b is not None:
                y_finish(b, 3)
            if f is not None:
                fc_gemm(f, 0)
            if a is not None:
                attn_pv(a)
            if f is not None:
                fc_gemm(f, 1)
            if g is not None and g < BPC - 1:
                store(g)
            if a is not None:
                attn_proto(a)
            if f is not None:
                fc_gemm(f, 2)
                fc_gemm(f, 3)


_PROGRAM_CACHE = {}


def _build(apply_gb):
    key = bool(apply_gb)
    if key in _PROGRAM_CACHE:
        return _PROGRAM_CACHE[key]
    nc = bacc.Bacc("TRN2", target_bir_lowering=False, debug=False,
                   num_devices=N_CORES)
    ext = {}
    ext["qv"] = nc.declare_dram_parameter("qv", [BPC, 2, LP, D], F8, isOutput=False)
    ext["kT"] = nc.declare_dram_parameter("kT", [BPC, D, LP], F16, isOutput=False)
    for name in ("wkT", "wfcT", "wvT", "wqk"):
        ext[name] = nc.declare_dram_parameter(name, [D, D], F16, isOutput=False)
    ext["sel"] = nc.declare_dram_parameter("sel", [LP, W], F8, isOutput=False)
    ext["ones"] = nc.declare_dram_parameter("ones", [LP, 1], F8, isOutput=False)
    ext["bc5"] = nc.declare_dram_parameter("bc5", [W, LP], F16, isOutput=False)
    ext["ident"] = nc.declare_dram_parameter("ident", [DT, DT], F16, isOutput=False)
    if apply_gb:
        for name in ("g1", "b1", "g2", "b2"):
            ext[name] = nc.declare_dram_parameter(name, [D], F32, isOutput=False)
    ext["out"] = nc.declare_dram_parameter("out", [BPC, LP, D], F16, isOutput=True)

    with tile.TileContext(nc) as tc:
        _emit(nc, tc, ext, apply_gb)
    nc.compile()
    _PROGRAM_CACHE[key] = (nc, apply_gb)
    return _PROGRAM_CACHE[key]


def kernel(q, k, v, Wq, Wk, Wv, Wfc, g1, b1, g2, b2, _trace=False):
    q = np.asarray(q, dtype=np.float32)
    k = np.asarray(k, dtype=np.float32)
    v = np.asarray(v, dtype=np.float32)
    Wq = np.asarray(Wq, dtype=np.float32)
    Wk = np.asarray(Wk, dtype=np.float32)
    Wv = np.asarray(Wv, dtype=np.float32)
    Wfc = np.asarray(Wfc, dtype=np.float32)
    g1 = np.asarray(g1, dtype=np.float32)
    b1 = np.asarray(b1, dtype=np.float32)
    g2 = np.asarray(g2, dtype=np.float32)
    b2 = np.asarray(b2, dtype=np.float32)

    apply_gb = not (np.all(g1 == 1) and np.all(b1 == 0)
                    and np.all(g2 == 1) and np.all(b2 == 0))

    f8 = ml_dtypes.float8_e4m3

    def pad8(x):
        out = np.zeros((B, LP, D), dtype=f8)
        out[:, :L, :] = x.astype(f8)
        return out

    qv8 = np.ascontiguousarray(np.stack([pad8(q), pad8(v)], axis=1))
    k16 = np.zeros((B, LP, D), dtype=np.float16)
    k16[:, :L, :] = k.astype(np.float16)
    kT16 = np.ascontiguousarray(k16.transpose(0, 2, 1))
    wkT = np.ascontiguousarray(Wk.T).astype(np.float16)
    wfcT = np.ascontiguousarray(Wfc.T).astype(np.float16)
    wvT = np.ascontiguousarray(Wv.T).astype(np.float16)
    wqk = ((Wq.T.astype(np.float64) @ Wk.astype(np.float64)) / TEMP).astype(np.float16)
    sel = np.zeros((LP, W), dtype=f8)
    sel[np.arange(L), np.arange(L) % W] = 1.0
    ones = np.zeros((LP, 1), dtype=f8)
    ones[:L] = 1.0
    bc5 = np.zeros((W, LP), dtype=np.float16)
    bc5[np.arange(L) % W, np.arange(L)] = 1.0
    ident = np.eye(DT, dtype=np.float16)

    nc, _ = _build(apply_gb)

    in_maps = []
    for c in range(N_CORES):
        m = {
            "qv": qv8[c * BPC:(c + 1) * BPC],
            "kT": kT16[c * BPC:(c + 1) * BPC],
            "wkT": wkT, "wfcT": wfcT, "wvT": wvT, "wqk": wqk,
            "sel": sel, "ones": ones, "bc5": bc5, "ident": ident,
        }
        if apply_gb:
            m.update({"g1": g1, "b1": b1, "g2": g2, "b2": b2})
        in_maps.append(m)

    res = run_bass_kernel_spmd(nc, in_maps, core_ids=list(range(N_CORES)),
                               trace=_trace)
    out = np.concatenate([res.results[c]["out"] for c in range(N_CORES)],
                         axis=0)[:, :L, :].astype(np.float32)
    if _trace:
        kernel._last_results = res
    return out


# revision 53
# speedup vs baseline: 1.7213x; 1.0190x over previous
"""Trainium2 Bass kernel for nn_MultiHeadAttention_18700287607660.

Math (B=128, L=500, D=512, NWAY=5, n_head=1):
  qp = q@Wq.T ; kp = k@Wk.T ; vp = v@Wv.T
  attn_avg = softmax(mean_over_groups(qp @ kp.T / temp))     # [B, 5, L]
  proto = attn_avg @ vp                                      # [B, 5, D]
  out1 = LN1(broadcast(proto) + kp)
  out  = LN2(leaky_relu(out1@Wfc.T, 0.1) + out1)

Restructurings (exact up to fp reassociation):
  * mean before softmax: S = (Sel@q) @ (Wq.T@Wk/temp) @ k.T, Wqk host-folded.
  * the whole attention chain is computed TRANSPOSED ([d, 5] layouts) so every
    matmul has a 5-wide moving dim: the PE cost model charges out-free-size
    cycles, so these are ~4ns each instead of ~213ns.  The softmax norm is
    deferred: sums come from a [5,1] ones-matmul, 1/sum is applied as a
    per-partition Act scale on the final [5,512] proto copy.
  * softmax max-subtraction dropped (logits ~N(0, 0.1^2) — exp is safe); the
    1/100 group-mean scale rides the exp's input scale.
  * proto'v broadcast over shots is a K=5 matmul accumulated into kp's PSUM.
  * leaky_relu is a single fused Prelu activation (alpha=0.1).
  * q, v stream in as fp8e4m3 (attention-side only — error lands on proto,
    which is ~5% the magnitude of kp); k stays fp16 and is transposed on the
    host so the kernel does a plain load instead of a DMA-xbar transpose.

Sharding: pure data parallel, 16 batches per core across 8 cores.
"""
import os
import sys

for _p in ("/opt/trn_rl_repo", "/root/.axon_site/_ro/trn_rl_repo"):
    if os.path.isdir(_p) and _p not in sys.path:
        sys.path.insert(0, _p)

import numpy as np
import ml_dtypes

import concourse.bacc as bacc
import concourse.bass as bass
import concourse.tile as tile
from concourse import mybir
from concourse.bass_utils import run_bass_kernel_spmd

F8 = mybir.dt.float8e4
F16 = mybir.dt.float16
F32 = mybir.dt.float32
ACT = mybir.ActivationFunctionType
N_CORES = 8
B = 128
BPC = B // N_CORES   # 16 batches per core
L = 500              # true seq len
LP = 512             # padded seq len
LT = 128             # l-tile
NLT = LP // LT       # 4
D = 512
DT = 128
NDT = D // DT        # 4
W = 5                # NWAY shot groups
TEMP = float(np.sqrt(float(D)))
NG = 100.0           # number of shot groups averaged (Lq // NWAY)
EPS = 1e-6
LEAK = 0.1

# All ACT functions used here (Exp, Ln, Prelu, Copy, Identity) live in the
# "natural_log_exp_and_others" table set, but bacc's per-activation greedy
# set chooser still flips between sets.  Empty out every other set (keeping
# positions, since act_func_set_id is the positional index) so exactly one
# set is ever loaded.
_orig_get_activation_tables = bacc.get_activation_tables


def _pinned_activation_tables(module_arch):
    tables = _orig_get_activation_tables(module_arch)
    if "natural_log_exp_and_others" in tables:
        return {
            name: (fns if name == "natural_log_exp_and_others" else set())
            for name, fns in tables.items()
        }
    return tables


bacc.get_activation_tables = _pinned_activation_tables


def _emit(nc, tc, ext, apply_gb):
    """Software-pipelined emission.  Iteration s emits
        store(s-2) | load(s+2) | attn(s+1) x y(s) x fc(s-1)
    with the attention chain's tiny matmuls interleaved between the big GEMM
    l-tiles so the in-order PE stream never waits on a cross-engine hop.
    """
    import contextlib
    ctx = contextlib.ExitStack()
    with ctx:
        const = ctx.enter_context(tc.tile_pool(name="const", bufs=1))
        pqv = ctx.enter_context(tc.tile_pool(name="pqv", bufs=4))
        pkt = ctx.enter_context(tc.tile_pool(name="pkt", bufs=4))
        px = ctx.enter_context(tc.tile_pool(name="px", bufs=5))
        pxt = ctx.enter_context(tc.tile_pool(name="pxt", bufs=4))
        pt = ctx.enter_context(tc.tile_pool(name="pt", bufs=3))
        pr = ctx.enter_context(tc.tile_pool(name="pr", bufs=3))
        po = ctx.enter_context(tc.tile_pool(name="po", bufs=2))
        tiny = ctx.enter_context(tc.tile_pool(name="tiny", bufs=3))
        ps_y = ctx.enter_context(tc.tile_pool(name="ps_y", bufs=5, space="PSUM"))
        ps_z = ctx.enter_context(tc.tile_pool(name="ps_z", bufs=2, space="PSUM"))
        ps_att = ctx.enter_context(tc.tile_pool(name="ps_att", bufs=1, space="PSUM"))

        # ---- constants (packed into 3 DMAs to cut startup DGE overhead) ----
        wall_sb = const.tile([DT, 4, NDT, D], F16)
        wqk_sb = wall_sb[:, 0, :, :]
        wkT_sb = wall_sb[:, 1, :, :]
        wvT_sb = wall_sb[:, 2, :, :]
        wfcT_sb = wall_sb[:, 3, :, :]
        sel8_sb = const.tile([LT, NLT, W + 1], F8)
        sel_sb = sel8_sb[:, :, :W]
        nc.sync.dma_start(out=sel8_sb,
                          in_=ext["sel8"].rearrange("(i p) w -> p i w", p=LT))
        cb_sb = const.tile([DT, DT + LP], F16)
        id_sb = cb_sb[:, :DT]
        bc5_sb = cb_sb[:W, DT:]
        eps_sb = const.tile([DT, 1], F32)
        nc.vector.memset(eps_sb, EPS)
        gb_sb = {}
        if apply_gb:
            for name in ("g1", "b1", "g2", "b2"):
                t = const.tile([LT, D], F32)
                src = ext[name]
                bcast = bass.AP(tensor=src.tensor, offset=src.offset,
                                ap=[[0, LT]] + list(src.ap))
                nc.sync.dma_start(out=t, in_=bcast)
                gb_sb[name] = t

        state = {}

        def load(b):
            st = state.setdefault(b, {})
            st["qv"] = pqv.tile([LT, 2, NLT, D], F8, tag="qv", name=f"qv{b}")
            st["kT"] = pkt.tile([DT, NDT, LP], F16, tag="kT", name=f"kT{b}")
            nc.gpsimd.dma_start(out=st["qv"],
                                in_=ext["qv"][b].rearrange("t (i p) d -> p t i d", p=LT))
            nc.sync.dma_start(out=st["kT"],
                              in_=ext["kT"][b].rearrange("(i p) l -> p i l", p=DT))

        # ---- attention chain, transposed; split into interleavable steps ----
        def attn_qbar(b):
            st = state[b]
            psA = ps_att.tile([DT, NDT, W], F32, tag="att")
            q = st["qv"][:, 0, :, :]
            for j in range(NDT):
                for lt in range(NLT):
                    nc.tensor.matmul(psA[:, j, :], lhsT=q[:, lt, j * DT:(j + 1) * DT],
                                     rhs=sel_sb[:, lt, :],
                                     start=(lt == 0), stop=(lt == NLT - 1))
            st["qbT"] = tiny.tile([DT, NDT, W], F16, tag="qbT", name=f"qbT{b}")
            nc.vector.tensor_copy(out=st["qbT"], in_=psA)

        def attn_qk(b):
            st = state[b]
            psB = ps_att.tile([DT, NDT, W], F32, tag="att")
            for j in range(NDT):
                for i in range(NDT):
                    nc.tensor.matmul(psB[:, j, :], lhsT=wqk_sb[:, i, j * DT:(j + 1) * DT],
                                     rhs=st["qbT"][:, i, :],
                                     start=(i == 0), stop=(i == NDT - 1))
            st["qkT"] = tiny.tile([DT, NDT, W], F16, tag="qkT", name=f"qkT{b}")
            nc.vector.tensor_copy(out=st["qkT"], in_=psB)

        def attn_S(b):
            st = state[b]
            psS = ps_att.tile([LT, NLT, W], F32, tag="att")
            for lt in range(NLT):
                for i in range(NDT):
                    nc.tensor.matmul(psS[:, lt, :],
                                     lhsT=st["kT"][:, i, lt * LT:(lt + 1) * LT],
                                     rhs=st["qkT"][:, i, :],
                                     start=(i == 0), stop=(i == NDT - 1))
            # E = exp(S_raw / NG); the 1/NG group-mean scale rides the act.
            st["E"] = tiny.tile([LT, NLT, W], F8, tag="E", name=f"E{b}")
            nc.scalar.activation(out=st["E"], in_=psS, func=ACT.Exp,
                                 bias=0.0, scale=1.0 / NG)

        def attn_sums(b):
            st = state[b]
            psSum = ps_att.tile([W, 1], F32, tag="att")
            for lt in range(NLT):
                nc.tensor.matmul(psSum, lhsT=st["E"][:, lt, :],
                                 rhs=sel8_sb[:, lt, W:W + 1],
                                 start=(lt == 0), stop=(lt == NLT - 1))
            st["rcp"] = tiny.tile([W, 1], F32, tag="rcp", name=f"rcp{b}")
            nc.vector.reciprocal(out=st["rcp"], in_=psSum)

        def attn_t1(b):
            st = state[b]
            psC = ps_att.tile([DT, NDT, W], F32, tag="att")
            v = st["qv"][:, 1, :, :]
            for j in range(NDT):
                for lt in range(NLT):
                    nc.tensor.matmul(psC[:, j, :], lhsT=v[:, lt, j * DT:(j + 1) * DT],
                                     rhs=st["E"][:, lt, :],
                                     start=(lt == 0), stop=(lt == NLT - 1))
            st["t1T"] = tiny.tile([DT, NDT, W], F16, tag="t1T", name=f"t1T{b}")
            nc.vector.tensor_copy(out=st["t1T"], in_=psC)

        def attn_pv(b):
            st = state[b]
            psD = ps_att.tile([DT, NDT, W], F32, tag="att")
            for j in range(NDT):
                for i in range(NDT):
                    nc.tensor.matmul(psD[:, j, :], lhsT=wvT_sb[:, i, j * DT:(j + 1) * DT],
                                     rhs=st["t1T"][:, i, :],
                                     start=(i == 0), stop=(i == NDT - 1))
            st["pvT"] = tiny.tile([DT, NDT, W], F16, tag="pvT", name=f"pvT{b}")
            nc.vector.tensor_copy(out=st["pvT"], in_=psD)

        def attn_proto(b):
            st = state[b]
            psP = ps_att.tile([W, D], F16, tag="att")
            for j in range(NDT):
                nc.tensor.transpose(psP[:, j * DT:(j + 1) * DT], st["pvT"][:, j, :], id_sb)
            st["protoV"] = tiny.tile([W, D], F16, tag="protoV", name=f"protoV{b}")
            nc.scalar.activation(out=st["protoV"], in_=psP, func=ACT.Copy,
                                 scale=st["rcp"])

        # ---- y = kp + bcast(proto); LN1 ----
        def y_gemm(b, lt):
            st = state[b]
            if lt == 0:
                st["psY"] = {}
                st["st1"] = tiny.tile([LT, NLT, 6], F32, tag="st1", name=f"st1_{b}")
                st["mv1"] = tiny.tile([LT, NLT, 2], F32, tag="mv1", name=f"mv1_{b}")
            psY = ps_y.tile([LT, D], F32, tag="y", name=f"y{b}_{lt}")
            st["psY"][lt] = psY
            for i in range(NDT):
                nc.tensor.matmul(psY, lhsT=st["kT"][:, i, lt * LT:(lt + 1) * LT],
                                 rhs=wkT_sb[:, i, :], start=(i == 0), stop=False)
            nc.tensor.matmul(psY, lhsT=bc5_sb[:, lt * LT:(lt + 1) * LT],
                             rhs=st["protoV"], start=False, stop=True)
            nc.vector.bn_stats(out=st["st1"][:, lt, :], in_=psY)
            nc.vector.bn_aggr(out=st["mv1"][:, lt, :], in_=st["st1"][:, lt, :])

        def y_finish(b, lt):
            # Per-l-tile LN1 tail: frees the y PSUM bank early — the cycle
            # "y(b,lt) gemm -> stats -> rstd -> apply1 -> free bank for
            # y(b+1,lt)" is the loop-carried constraint that paces the whole
            # pipeline, so it must not wait for the other l-tiles' stats.
            st = state[b]
            mv1 = st["mv1"]
            if lt == 0:
                st["u1"] = tiny.tile([LT, NLT], F32, tag="u1", name=f"u1_{b}")
                st["rstd1"] = tiny.tile([LT, NLT], F32, tag="rstd1", name=f"rstd1_{b}")
                st["nb1"] = tiny.tile([LT, NLT], F32, tag="nb1", name=f"nb1_{b}")
                st["x"] = px.tile([LT, NLT, D], F16, tag="x", name=f"x{b}")
                st["xT"] = pxt.tile([DT, NLT, NDT, LT], F16, tag="xT", name=f"xT{b}")
            u1, rstd1, nb1 = st["u1"], st["rstd1"], st["nb1"]
            nc.scalar.activation(out=u1[:, lt:lt + 1], in_=mv1[:, lt, 1:2],
                                 func=ACT.Ln, bias=eps_sb, scale=1.0)
            nc.scalar.activation(out=rstd1[:, lt:lt + 1], in_=u1[:, lt:lt + 1],
                                 func=ACT.Exp, bias=0.0, scale=-0.5)
            nc.vector.scalar_tensor_tensor(out=nb1[:, lt:lt + 1],
                                           in0=mv1[:, lt, 0:1], scalar=-1.0,
                                           in1=rstd1[:, lt:lt + 1],
                                           op0=mybir.AluOpType.mult,
                                           op1=mybir.AluOpType.mult)
            nc.scalar.activation(out=st["x"][:, lt, :], in_=st["psY"][lt],
                                 func=ACT.Identity,
                                 bias=nb1[:, lt:lt + 1], scale=rstd1[:, lt:lt + 1])
            if apply_gb:
                nc.vector.tensor_mul(out=st["x"][:, lt, :], in0=st["x"][:, lt, :],
                                     in1=gb_sb["g1"])
                nc.vector.tensor_add(out=st["x"][:, lt, :], in0=st["x"][:, lt, :],
                                     in1=gb_sb["b1"])
            del st["psY"][lt]
            if b == BPC - 1 and lt in (1, NLT - 1):
                # tail: nothing left to overlap, so ship xT in halves to let
                # the last fc start sooner.
                nc.sync.dma_start_transpose(out=st["xT"][:, lt - 1:lt + 1, :, :],
                                            in_=st["x"][:, lt - 1:lt + 1, :])
            elif b != BPC - 1 and lt == NLT - 1:
                # one xbar transpose for the whole batch: [128, 2048] in and
                # out are fully contiguous with this xT layout.
                nc.sync.dma_start_transpose(out=st["xT"], in_=st["x"])

        # ---- fc + leaky + residual; LN2 ----
        def fc_gemm(b, lt):
            st = state[b]
            if lt == 0:
                st["t"] = pt.tile([LT, NLT, D], F16, tag="t", name=f"t{b}")
                st["r"] = pr.tile([LT, NLT, D], F16, tag="r", name=f"r{b}")
                st["st2"] = tiny.tile([LT, NLT, 6], F32, tag="st2", name=f"st2_{b}")
                st["mv2"] = tiny.tile([LT, NLT, 2], F32, tag="mv2", name=f"mv2_{b}")
            psZ = ps_z.tile([LT, D], F32, tag="z")
            for i in range(NDT):
                nc.tensor.matmul(psZ, lhsT=st["xT"][:, lt, i, :],
                                 rhs=wfcT_sb[:, i, :],
                                 start=(i == 0), stop=(i == NDT - 1))
            nc.scalar.activation(out=st["t"][:, lt, :], in_=psZ, func=ACT.Prelu,
                                 alpha=LEAK)
            eng = nc.vector if (lt % 2 == 0 or b == BPC - 1) else nc.gpsimd
            eng.tensor_add(out=st["r"][:, lt, :], in0=st["t"][:, lt, :],
                           in1=st["x"][:, lt, :])
            nc.vector.bn_stats(out=st["st2"][:, lt, :], in_=st["r"][:, lt, :])
            nc.vector.bn_aggr(out=st["mv2"][:, lt, :], in_=st["st2"][:, lt, :])
            if b == BPC - 1:
                # tail: telescoped per-l-tile LN2 finish + store — nothing
                # overlaps the last batch, so latency is all that matters.
                if lt == 0:
                    st["u2"] = tiny.tile([LT, NLT], F32, tag="u2t", name="u2_tail")
                    st["rstd2"] = tiny.tile([LT, NLT], F32, tag="rstd2t",
                                            name="rstd2_tail")
                    st["o"] = po.tile([LT, NLT, D], F16, tag="o", name=f"o{b}")
                nc.scalar.activation(out=st["u2"][:, lt:lt + 1],
                                     in_=st["mv2"][:, lt, 1:2],
                                     func=ACT.Ln, bias=eps_sb, scale=1.0)
                nc.scalar.activation(out=st["rstd2"][:, lt:lt + 1],
                                     in_=st["u2"][:, lt:lt + 1],
                                     func=ACT.Exp, bias=0.0, scale=-0.5)
                nc.vector.tensor_scalar(out=st["o"][:, lt, :],
                                        in0=st["r"][:, lt, :],
                                        scalar1=st["mv2"][:, lt, 0:1],
                                        scalar2=st["rstd2"][:, lt:lt + 1],
                                        op0=mybir.AluOpType.subtract,
                                        op1=mybir.AluOpType.mult)
                if apply_gb:
                    nc.vector.tensor_mul(out=st["o"][:, lt, :],
                                         in0=st["o"][:, lt, :], in1=gb_sb["g2"])
                    nc.vector.tensor_add(out=st["o"][:, lt, :],
                                         in0=st["o"][:, lt, :], in1=gb_sb["b2"])
                nc.sync.dma_start(
                    out=ext["out"][b].rearrange("(i p) d -> p i d", p=LT)[:, lt, :],
                    in_=st["o"][:, lt, :])

        def fc_finish(b):
            st = state[b]
            mv2 = st["mv2"]
            u2 = tiny.tile([LT, NLT], F32, tag="u2")
            rstd2 = tiny.tile([LT, NLT], F32, tag="rstd2")
            nc.scalar.activation(out=u2, in_=mv2[:, :, 1], func=ACT.Ln,
                                 bias=eps_sb, scale=1.0)
            nc.scalar.activation(out=rstd2, in_=u2, func=ACT.Exp,
                                 bias=0.0, scale=-0.5)
            o = po.tile([LT, NLT, D], F16, tag="o", name=f"o{b}")
            for lt in range(NLT):
                nc.vector.tensor_scalar(out=o[:, lt, :], in0=st["r"][:, lt, :],
                                        scalar1=mv2[:, lt, 0:1],
                                        scalar2=rstd2[:, lt:lt + 1],
                                        op0=mybir.AluOpType.subtract,
                                        op1=mybir.AluOpType.mult)
                if apply_gb:
                    nc.vector.tensor_mul(out=o[:, lt, :], in0=o[:, lt, :],
                                         in1=gb_sb["g2"])
                    nc.vector.tensor_add(out=o[:, lt, :], in0=o[:, lt, :],
                                         in1=gb_sb["b2"])
            st["o"] = o

        def store(b):
            eng = nc.sync if b == BPC - 1 else nc.gpsimd
            eng.dma_start(out=ext["out"][b].rearrange("(i p) d -> p i d", p=LT),
                          in_=state[b]["o"])
            del state[b]

        # ---- pipelined emission ----
        load(0)
        nc.sync.dma_start(out=wall_sb[:, 0, :, :],
                          in_=ext["wall"][0].rearrange("(i p) e -> p i e", p=DT))
        nc.sync.dma_start(out=wall_sb[:, 1:, :, :],
                          in_=ext["wall"][1:].rearrange("w (i p) e -> p w i e", p=DT))
        nc.sync.dma_start(out=cb_sb, in_=ext["cb"][:])
        load(1)
        attn_qbar(0)
        attn_qk(0)
        attn_S(0)
        attn_sums(0)
        attn_t1(0)
        attn_pv(0)
        attn_proto(0)
        for s in range(BPC + 3):
            a = s + 1 if s + 1 < BPC else None
            b = s if s < BPC else None
            f = s - 2 if 2 <= s <= BPC + 1 else None
            g = s - 3 if s >= 3 else None
            if a is not None:
                attn_qbar(a)
            if g is not None and g < BPC - 1:
                fc_finish(g)
            if s + 2 < BPC:
                load(s + 2)
            if b is not None:
                y_gemm(b, 0)
                y_finish(b, 0)
            if a is not None:
                attn_qk(a)
            if b is not None:
                y_gemm(b, 1)
            if a is not None:
                attn_S(a)
            if b is not None:
                y_finish(b, 1)
                y_gemm(b, 2)
            if a is not None:
                attn_sums(a)
            if b is not None:
                y_finish(b, 2)
                y_gemm(b, 3)
            if a is not None:
                attn_t1(a)
            if b is not None:
                y_finish(b, 3)
            if f is not None:
                fc_gemm(f, 0)
            if a is not None:
                attn_pv(a)
            if f is not None:
                fc_gemm(f, 1)
            if g is not None and g < BPC - 1:
                store(g)
            if a is not None:
                attn_proto(a)
            if f is not None:
                fc_gemm(f, 2)
                fc_gemm(f, 3)


_PROGRAM_CACHE = {}


def _build(apply_gb):
    key = bool(apply_gb)
    if key in _PROGRAM_CACHE:
        return _PROGRAM_CACHE[key]
    nc = bacc.Bacc("TRN2", target_bir_lowering=False, debug=False,
                   num_devices=N_CORES)
    ext = {}
    ext["qv"] = nc.declare_dram_parameter("qv", [BPC, 2, LP, D], F8, isOutput=False)
    ext["kT"] = nc.declare_dram_parameter("kT", [BPC, D, LP], F16, isOutput=False)
    ext["wall"] = nc.declare_dram_parameter("wall", [4, D, D], F16, isOutput=False)
    ext["sel8"] = nc.declare_dram_parameter("sel8", [LP, W + 1], F8, isOutput=False)
    ext["cb"] = nc.declare_dram_parameter("cb", [DT, DT + LP], F16, isOutput=False)
    if apply_gb:
        for name in ("g1", "b1", "g2", "b2"):
            ext[name] = nc.declare_dram_parameter(name, [D], F32, isOutput=False)
    ext["out"] = nc.declare_dram_parameter("out", [BPC, LP, D], F16, isOutput=True)

    with tile.TileContext(nc) as tc:
        _emit(nc, tc, ext, apply_gb)
    nc.compile()
    _PROGRAM_CACHE[key] = (nc, apply_gb)
    return _PROGRAM_CACHE[key]


def kernel(q, k, v, Wq, Wk, Wv, Wfc, g1, b1, g2, b2, _trace=False):
    q = np.asarray(q, dtype=np.float32)
    k = np.asarray(k, dtype=np.float32)
    v = np.asarray(v, dtype=np.float32)
    Wq = np.asarray(Wq, dtype=np.float32)
    Wk = np.asarray(Wk, dtype=np.float32)
    Wv = np.asarray(Wv, dtype=np.float32)
    Wfc = np.asarray(Wfc, dtype=np.float32)
    g1 = np.asarray(g1, dtype=np.float32)
    b1 = np.asarray(b1, dtype=np.float32)
    g2 = np.asarray(g2, dtype=np.float32)
    b2 = np.asarray(b2, dtype=np.float32)

    apply_gb = not (np.all(g1 == 1) and np.all(b1 == 0)
                    and np.all(g2 == 1) and np.all(b2 == 0))

    f8 = ml_dtypes.float8_e4m3

    def pad8(x):
        out = np.zeros((B, LP, D), dtype=f8)
        out[:, :L, :] = x.astype(f8)
        return out

    qv8 = np.ascontiguousarray(np.stack([pad8(q), pad8(v)], axis=1))
    k16 = np.zeros((B, LP, D), dtype=np.float16)
    k16[:, :L, :] = k.astype(np.float16)
    kT16 = np.ascontiguousarray(k16.transpose(0, 2, 1))
    wqk = ((Wq.T.astype(np.float64) @ Wk.astype(np.float64)) / TEMP).astype(np.float16)
    wall = np.ascontiguousarray(np.stack([
        wqk, Wk.T.astype(np.float16), Wv.T.astype(np.float16),
        Wfc.T.astype(np.float16)]))
    sel8 = np.zeros((LP, W + 1), dtype=f8)
    sel8[np.arange(L), np.arange(L) % W] = 1.0
    sel8[:L, W] = 1.0
    cb = np.zeros((DT, DT + LP), dtype=np.float16)
    cb[:, :DT] = np.eye(DT, dtype=np.float16)
    cb[np.arange(L) % W, DT + np.arange(L)] = 1.0

    nc, _ = _build(apply_gb)

    in_maps = []
    for c in range(N_CORES):
        m = {
            "qv": qv8[c * BPC:(c + 1) * BPC],
            "kT": kT16[c * BPC:(c + 1) * BPC],
            "wall": wall, "sel8": sel8, "cb": cb,
        }
        if apply_gb:
            m.update({"g1": g1, "b1": b1, "g2": g2, "b2": b2})
        in_maps.append(m)

    res = run_bass_kernel_spmd(nc, in_maps, core_ids=list(range(N_CORES)),
                               trace=_trace)
    out = np.concatenate([res.results[c]["out"] for c in range(N_CORES)],
                         axis=0)[:, :L, :].astype(np.float32)
    if _trace:
        kernel._last_results = res
    return out
